# revision 20
# baseline (speedup 1.0000x reference)
"""Bass/Trainium2 SPMD kernel for nn_DSSKernel (DSS: Diagonal State Space kernel).

K[c,h,l] = Re( sum_n Wc'[c,h,n] * exp(dt_Lambda[h,n] * (l - s[n])) ),  c=C=1

Strategy:
 - Shard H=512 across 8 cores (Hc=64 per core). No cross-core comms.
 - Split l = q*T + r (T=128, Q=16): S = U (x) V needs only N*Hc*(T+Q)
   transcendentals per core instead of N*Hc*L (14x fewer).
 - Per-core layout: partition p in [0,128): n = p%64, j = p//64 (h parity),
   h = 2m+j for free-dim index m in [0,32). Every lane does unique work.
   Input layout transforms (transpose/broadcast/shard) happen host-side, so
   every device DMA is contiguous.
 - Contraction over n via PE: per h-pair a PSUM-accumulated pair of f32r
   matmuls with block-diagonal stationary operands; 4 pairs pack one PSUM
   tile at partition offsets.
 - sin via ACT (valid range [-pi,pi]) with magic-number round + Cody-Waite
   range reduction; cos(x) = sin(wrap(x + pi/2)).
 - ACT calls chained exp-batch -> sin-batch (2 table loads total).
"""

import sys

import numpy as np

if "/opt/trn_rl_repo" not in sys.path:
    sys.path.insert(0, "/opt/trn_rl_repo")

import concourse.bacc as bacc
import concourse.bass as bass
import concourse.tile as tile
from concourse import mybir
from concourse.tile import add_dep_helper

f32 = mybir.dt.float32
f32r = mybir.dt.float32r
Act = mybir.ActivationFunctionType
Alu = mybir.AluOpType

M_CORES = 8
H, N, L = 512, 64, 2048
HC = H // M_CORES          # 64 h-channels per core
T = 128                    # inner block length (V)
Q = L // T                 # 16 outer blocks (U)
MH = HC // 2               # 32 h-pairs per core
PB = 2                     # h-pairs packed per PSUM tile (PSUM base partition must be 0/32/64)
EPS = 1e-7

PI = float(np.pi)
HALF_PI = float(np.pi / 2)
TWO_PI = float(2 * np.pi)
INV_2PI = float(np.float32(1.0 / (2 * np.pi)))
MAGIC = 12582912.0         # 1.5 * 2^23: round-to-nearest for |y| < 2^22
C1 = 6.28125
_rem = 2 * np.pi - np.float64(np.float32(C1))
C2 = float(np.float32(_rem))
C3 = float(np.float32(_rem - np.float64(np.float32(_rem))))


def _ap(t, offset, pattern):
    return bass.AP(tensor=t, offset=offset, ap=[list(p) for p in pattern])


def prep_core_inputs(c, log_dt, Lambda, W):
    """Host-side shard + layout: partition p = 64*j + n, free m; h = 2m+j."""
    hs = slice(c * HC, (c + 1) * HC)
    Wc = np.asarray(W, np.float32)[0, hs]            # (HC, N, 2)
    ld = np.asarray(log_dt, np.float32)[hs]          # (HC, 2)
    lam = np.asarray(Lambda, np.float32)             # (N, 2)
    # W[2m+j, n, comp] -> [64j+n, m]
    wre = Wc[:, :, 0].reshape(MH, 2, N).transpose(1, 2, 0).reshape(128, MH)
    wim = Wc[:, :, 1].reshape(MH, 2, N).transpose(1, 2, 0).reshape(128, MH)
    # log_dt[2m+j, comp] broadcast over n -> [64j+n, m]
    ldr = np.broadcast_to(
        ld[:, 0].reshape(MH, 2).T[:, None, :], (2, N, MH)
    ).reshape(128, MH)
    ldi = np.broadcast_to(
        ld[:, 1].reshape(MH, 2).T[:, None, :], (2, N, MH)
    ).reshape(128, MH)
    return {
        "Wre": np.ascontiguousarray(wre),
        "Wim": np.ascontiguousarray(wim),
        "ldt_re": np.ascontiguousarray(ldr),
        "ldt_im": np.ascontiguousarray(ldi),
        "Lam": np.ascontiguousarray(np.tile(lam, (2, 1))),  # (128, 2)
    }


def build_kernel():
    nc = bacc.Bacc()
    in_wre = nc.dram_tensor("Wre", [128, MH], f32, kind="ExternalInput")
    in_wim = nc.dram_tensor("Wim", [128, MH], f32, kind="ExternalInput")
    in_ldr = nc.dram_tensor("ldt_re", [128, MH], f32, kind="ExternalInput")
    in_ldi = nc.dram_tensor("ldt_im", [128, MH], f32, kind="ExternalInput")
    in_lam = nc.dram_tensor("Lam", [128, 2], f32, kind="ExternalInput")
    K = nc.dram_tensor("K", [HC, L], f32, kind="ExternalOutput")

    exps = []  # ACT order: all exps first, then all sins (2 table loads)
    sins = []

    def _last(lst):
        lst.append(list(nc.all_instructions())[-1])

    with tile.TileContext(nc) as tc:
        with (
            tc.tile_pool(name="prep", bufs=1) as prep,
            tc.tile_pool(name="big", bufs=1) as big,
            tc.tile_pool(name="psum", bufs=8, space="PSUM") as psum,
            tc.tile_pool(name="stg", bufs=6) as stg,
        ):
            P = 128

            def v3(t, inner):
                return t[:].rearrange("p (m x) -> p m x", x=inner)

            # ---------------- input loads (all contiguous) ----------------
            lam_sb = prep.tile([P, 2], f32, tag="lam")
            nc.sync.dma_start(out=lam_sb[:], in_=in_lam[:, :])
            lam_re = lam_sb[:, 0:1]
            lam_im = lam_sb[:, 1:2]
            ldt_re = prep.tile([P, MH], f32, tag="ldt_re")
            ldt_im = prep.tile([P, MH], f32, tag="ldt_im")
            nc.sync.dma_start(out=ldt_re[:], in_=in_ldr[:, :])
            nc.sync.dma_start(out=ldt_im[:], in_=in_ldi[:, :])
            w_re = prep.tile([P, MH], f32, tag="w_re")
            w_im = prep.tile([P, MH], f32, tag="w_im")
            nc.sync.dma_start(out=w_re[:], in_=in_wre[:, :])
            nc.sync.dma_start(out=w_im[:], in_=in_wim[:, :])

            # ---------------- phase A: per-(n,h) scalars [P, MH] ----------------
            dt_re = prep.tile([P, MH], f32, tag="dt_re")
            dt_im = prep.tile([P, MH], f32, tag="dt_im")
            nc.scalar.activation(dt_re[:], ldt_re[:], Act.Exp)
            _last(exps)
            nc.scalar.activation(dt_im[:], ldt_im[:], Act.Exp)
            _last(exps)

            a_re = prep.tile([P, MH], f32, tag="a_re")
            a_im = prep.tile([P, MH], f32, tag="a_im")
            nc.vector.tensor_scalar_mul(a_re[:], dt_re[:], lam_re)
            nc.vector.tensor_scalar_mul(a_im[:], dt_im[:], lam_im)

            pos = prep.tile([P, 1], f32, tag="pos")
            s1 = prep.tile([P, 1], f32, tag="s1")
            sshift = prep.tile([P, 1], f32, tag="sshift")
            nc.vector.tensor_scalar(pos[:], lam_re, 0.0, None, Alu.is_gt)
            nc.vector.tensor_scalar(s1[:], pos[:], -2.0, 1.0, Alu.mult, Alu.add)
            nc.vector.tensor_scalar_mul(sshift[:], pos[:], float(L - 1))

            an_re = prep.tile([P, MH], f32, tag="an_re")
            an_im = prep.tile([P, MH], f32, tag="an_im")
            nc.vector.tensor_scalar_mul(an_re[:], a_re[:], s1[:])
            nc.vector.tensor_scalar_mul(an_im[:], a_im[:], s1[:])

            e1 = prep.tile([P, MH], f32, tag="e1")
            nc.scalar.activation(e1[:], an_re[:], Act.Exp)
            _last(exps)
            eL = prep.tile([P, MH], f32, tag="eL")
            nc.scalar.activation(eL[:], an_re[:], Act.Exp, scale=float(L))
            _last(exps)

            def reduce_phase(theta_ap, out_t, k_t, t1_t):
                # k = round(theta/2pi) via magic number; out = cody_waite(theta, k)
                nc.vector.tensor_scalar(
                    t1_t[:], theta_ap, INV_2PI, MAGIC, Alu.mult, Alu.add
                )
                nc.vector.tensor_scalar(k_t[:], t1_t[:], MAGIC, None, Alu.subtract)
                nc.vector.cody_waite_cascade(out_t[:], theta_ap, k_t[:], C1, C2, C3)

            # th1 = an_im mod 2pi ; used for num and (scaled by L) for den
            th1 = prep.tile([P, MH], f32, tag="th1")
            tmp1 = prep.tile([P, MH], f32, tag="tmp1")
            tmp2 = prep.tile([P, MH], f32, tag="tmp2")
            reduce_phase(an_im[:], th1, tmp1, tmp2)
            th1c = prep.tile([P, MH], f32, tag="th1c")
            nc.vector.add_range_wrap(th1c[:], th1[:], HALF_PI, PI, TWO_PI)
            nc.vector.add_range_wrap(th1[:], th1[:], 0.0, PI, TWO_PI)

            # phiL = (an_im mod 2pi) * L, reduced again
            phiL = prep.tile([P, MH], f32, tag="phiL")
            nc.vector.tensor_scalar_mul(phiL[:], th1[:], float(L))
            thL = prep.tile([P, MH], f32, tag="thL")
            reduce_phase(phiL[:], thL, tmp1, tmp2)
            thLc = prep.tile([P, MH], f32, tag="thLc")
            nc.vector.add_range_wrap(thLc[:], thL[:], HALF_PI, PI, TWO_PI)
            nc.vector.add_range_wrap(thL[:], thL[:], 0.0, PI, TWO_PI)

            sin1 = prep.tile([P, MH], f32, tag="sin1")
            cos1 = prep.tile([P, MH], f32, tag="cos1")
            sinL = prep.tile([P, MH], f32, tag="sinL")
            cosL = prep.tile([P, MH], f32, tag="cosL")
            nc.scalar.activation(sin1[:], th1[:], Act.Sin)
            _last(sins)
            nc.scalar.activation(cos1[:], th1c[:], Act.Sin)
            _last(sins)
            nc.scalar.activation(sinL[:], thL[:], Act.Sin)
            _last(sins)
            nc.scalar.activation(cosL[:], thLc[:], Act.Sin)
            _last(sins)

            # num = e1*(cos1 + i sin1) - 1 ; den = eL*(cosL + i sinL) - 1
            num_re = prep.tile([P, MH], f32, tag="num_re")
            num_im = prep.tile([P, MH], f32, tag="num_im")
            nc.vector.tensor_mul(num_re[:], e1[:], cos1[:])
            nc.vector.tensor_scalar(num_re[:], num_re[:], 1.0, None, Alu.subtract)
            nc.vector.tensor_mul(num_im[:], e1[:], sin1[:])
            den_re = prep.tile([P, MH], f32, tag="den_re")
            den_im = prep.tile([P, MH], f32, tag="den_im")
            nc.vector.tensor_mul(den_re[:], eL[:], cosL[:])
            nc.vector.tensor_scalar(den_re[:], den_re[:], 1.0, None, Alu.subtract)
            nc.vector.tensor_mul(den_im[:], eL[:], sinL[:])

            # x = den * Lam ; recip = conj(x)/(|x|^2 + eps) = rr - i*ri
            x_re = prep.tile([P, MH], f32, tag="x_re")
            x_im = prep.tile([P, MH], f32, tag="x_im")
            nc.vector.tensor_scalar_mul(x_re[:], den_re[:], lam_re)
            nc.vector.tensor_scalar_mul(tmp1[:], den_im[:], lam_im)
            nc.vector.tensor_sub(x_re[:], x_re[:], tmp1[:])
            nc.vector.tensor_scalar_mul(x_im[:], den_re[:], lam_im)
            nc.vector.tensor_scalar_mul(tmp1[:], den_im[:], lam_re)
            nc.vector.tensor_add(x_im[:], x_im[:], tmp1[:])

            d2 = prep.tile([P, MH], f32, tag="d2")
            nc.vector.tensor_mul(d2[:], x_re[:], x_re[:])
            nc.vector.tensor_mul(tmp1[:], x_im[:], x_im[:])
            nc.vector.tensor_add(d2[:], d2[:], tmp1[:])
            nc.vector.tensor_scalar(d2[:], d2[:], EPS, None, Alu.add)
            inv = prep.tile([P, MH], f32, tag="inv")
            nc.vector.reciprocal(inv[:], d2[:])
            rr = prep.tile([P, MH], f32, tag="rr")
            ri = prep.tile([P, MH], f32, tag="ri")
            nc.vector.tensor_mul(rr[:], x_re[:], inv[:])
            nc.vector.tensor_mul(ri[:], x_im[:], inv[:])

            # F = num * (rr - i*ri):  F_re = num_re*rr + num_im*ri
            #                         F_im = num_im*rr - num_re*ri
            f_re = prep.tile([P, MH], f32, tag="f_re")
            f_im = prep.tile([P, MH], f32, tag="f_im")
            nc.vector.tensor_mul(f_re[:], num_re[:], rr[:])
            nc.vector.tensor_mul(tmp1[:], num_im[:], ri[:])
            nc.vector.tensor_add(f_re[:], f_re[:], tmp1[:])
            nc.vector.tensor_mul(f_im[:], num_im[:], rr[:])
            nc.vector.tensor_mul(tmp1[:], num_re[:], ri[:])
            nc.vector.tensor_sub(f_im[:], f_im[:], tmp1[:])

            # B = Wc * F
            b_re = prep.tile([P, MH], f32, tag="b_re")
            b_im = prep.tile([P, MH], f32, tag="b_im")
            nc.vector.tensor_mul(b_re[:], w_re[:], f_re[:])
            nc.vector.tensor_mul(tmp1[:], w_im[:], f_im[:])
            nc.vector.tensor_sub(b_re[:], b_re[:], tmp1[:])
            nc.vector.tensor_mul(b_im[:], w_re[:], f_im[:])
            nc.vector.tensor_mul(tmp1[:], w_im[:], f_re[:])
            nc.vector.tensor_add(b_im[:], b_im[:], tmp1[:])

            # pre-reduced A_im (integer multiples preserve phase mod 2pi)
            a_imr = prep.tile([P, MH], f32, tag="a_imr")
            reduce_phase(a_im[:], a_imr, tmp1, tmp2)

            # ---------------- V build: [P, MH*T] ----------------
            iota_vf = big.tile([P, MH * T], f32, tag="vE")
            nc.gpsimd.iota(
                iota_vf[:], pattern=[[0, MH], [1, T]], channel_multiplier=0,
                allow_small_or_imprecise_dtypes=True,
            )
            arg_re = big.tile([P, MH * T], f32, tag="vB")
            nc.vector.tensor_tensor(
                v3(arg_re, T), v3(iota_vf, T),
                a_re[:, :, None].broadcast_to((P, MH, T)), Alu.mult
            )
            ev = big.tile([P, MH * T], f32, tag="vF")
            nc.scalar.activation(ev[:], arg_re[:], Act.Exp)
            _last(exps)

            arg_im = big.tile([P, MH * T], f32, tag="vD")
            nc.vector.tensor_tensor(
                v3(arg_im, T), v3(iota_vf, T),
                a_imr[:, :, None].broadcast_to((P, MH, T)), Alu.mult
            )
            v_t1 = big.tile([P, MH * T], f32, tag="vA")
            v_k = big.tile([P, MH * T], f32, tag="vC")
            nc.vector.tensor_scalar(v_t1[:], arg_im[:], INV_2PI, MAGIC, Alu.mult, Alu.add)
            nc.vector.tensor_scalar(v_k[:], v_t1[:], MAGIC, None, Alu.subtract)
            th_v = big.tile([P, MH * T], f32, tag="vH")
            nc.vector.cody_waite_cascade(th_v[:], arg_im[:], v_k[:], C1, C2, C3)
            thc_v = big.tile([P, MH * T], f32, tag="vB")
            nc.vector.add_range_wrap(thc_v[:], th_v[:], HALF_PI, PI, TWO_PI)
            nc.vector.add_range_wrap(th_v[:], th_v[:], 0.0, PI, TWO_PI)

            scos_v = big.tile([P, MH * T], f32, tag="vC")
            ssin_v = big.tile([P, MH * T], f32, tag="vD")
            nc.scalar.activation(scos_v[:], thc_v[:], Act.Sin)
            _last(sins)
            nc.scalar.activation(ssin_v[:], th_v[:], Act.Sin)
            _last(sins)

            v_re = big.tile([P, MH * T], f32r, tag="vE")
            v_im = big.tile([P, MH * T], f32r, tag="vG")
            nc.vector.tensor_mul(v_re[:], ev[:], scos_v[:])
            nc.vector.tensor_mul(v_im[:], ev[:], ssin_v[:])

            # ---------------- U build: [P, MH*Q] ----------------
            iota_uf = big.tile([P, MH * Q], f32, tag="iota_uf")
            nc.gpsimd.iota(
                iota_uf[:], pattern=[[0, MH], [T, Q]], channel_multiplier=0,
                allow_small_or_imprecise_dtypes=True,
            )
            tq_s = big.tile([P, MH * Q], f32, tag="tq_s")
            nc.vector.tensor_scalar(tq_s[:], iota_uf[:], sshift[:], None, Alu.subtract)

            u_arg_re = big.tile([P, MH * Q], f32, tag="u_arg_re")
            nc.vector.tensor_tensor(
                v3(u_arg_re, Q), v3(tq_s, Q),
                a_re[:, :, None].broadcast_to((P, MH, Q)), Alu.mult
            )
            eu = big.tile([P, MH * Q], f32, tag="u_eu")
            nc.scalar.activation(eu[:], u_arg_re[:], Act.Exp)
            _last(exps)

            u_arg_im = big.tile([P, MH * Q], f32, tag="u_arg_im")
            nc.vector.tensor_tensor(
                v3(u_arg_im, Q), v3(tq_s, Q),
                a_imr[:, :, None].broadcast_to((P, MH, Q)), Alu.mult
            )
            u_t1 = big.tile([P, MH * Q], f32, tag="u_t1")
            u_k = big.tile([P, MH * Q], f32, tag="u_k")
            nc.vector.tensor_scalar(u_t1[:], u_arg_im[:], INV_2PI, MAGIC, Alu.mult, Alu.add)
            nc.vector.tensor_scalar(u_k[:], u_t1[:], MAGIC, None, Alu.subtract)
            th_u = big.tile([P, MH * Q], f32, tag="u_th")
            nc.vector.cody_waite_cascade(th_u[:], u_arg_im[:], u_k[:], C1, C2, C3)
            thc_u = big.tile([P, MH * Q], f32, tag="u_thc")
            nc.vector.add_range_wrap(thc_u[:], th_u[:], HALF_PI, PI, TWO_PI)
            nc.vector.add_range_wrap(th_u[:], th_u[:], 0.0, PI, TWO_PI)

            scos_u = big.tile([P, MH * Q], f32, tag="u_scos")
            ssin_u = big.tile([P, MH * Q], f32, tag="u_ssin")
            nc.scalar.activation(scos_u[:], thc_u[:], Act.Sin)
            _last(sins)
            nc.scalar.activation(ssin_u[:], th_u[:], Act.Sin)
            _last(sins)

            ec = big.tile([P, MH * Q], f32, tag="u_ec")
            es = big.tile([P, MH * Q], f32, tag="u_es")
            nc.vector.tensor_mul(ec[:], eu[:], scos_u[:])
            nc.vector.tensor_mul(es[:], eu[:], ssin_u[:])

            # Assemble block-diagonal stationary tiles directly:
            # lhs_top rows j -> cols jQ..jQ+Q-1 = U_top = B_re*EC - B_im*ES
            # lhs_bot rows j -> cols jQ..jQ+Q-1 = -(B_re*ES + B_im*EC)
            lhs_top = big.tile([P, MH * 2 * Q], f32r, tag="lhs_top")
            lhs_bot = big.tile([P, MH * 2 * Q], f32r, tag="lhs_bot")
            nc.vector.memset(lhs_top[:].bitcast(f32), 0.0)
            nc.vector.memset(lhs_bot[:].bitcast(f32), 0.0)
            lhs_top3 = v3(lhs_top, 2 * Q)
            lhs_bot3 = v3(lhs_bot, 2 * Q)
            ec3 = v3(ec, Q)
            es3 = v3(es, Q)
            t_a = big.tile([P, MH * Q], f32, tag="t_a")
            t_b = big.tile([P, MH * Q], f32, tag="t_b")
            t_a3 = v3(t_a, Q)
            t_b3 = v3(t_b, Q)
            # t_a = B_im*ES ; t_b = B_re*EC ; top = t_b - t_a
            nc.vector.tensor_tensor(
                t_a3, es3, b_im[:, :, None].broadcast_to((P, MH, Q)), Alu.mult
            )
            nc.vector.tensor_tensor(
                t_b3, ec3, b_re[:, :, None].broadcast_to((P, MH, Q)), Alu.mult
            )
            for jj in range(2):
                sl = slice(jj * 64, (jj + 1) * 64)
                cr = slice(jj * Q, (jj + 1) * Q)
                nc.vector.tensor_sub(lhs_top3[sl, :, cr], t_b3[sl], t_a3[sl])
            # t_a = B_re*ES ; t_b = B_im*EC ; bot = -t_a - t_b
            nc.vector.tensor_tensor(
                t_a3, es3, b_re[:, :, None].broadcast_to((P, MH, Q)), Alu.mult
            )
            nc.vector.tensor_tensor(
                t_b3, ec3, b_im[:, :, None].broadcast_to((P, MH, Q)), Alu.mult
            )
            for jj in range(2):
                sl = slice(jj * 64, (jj + 1) * 64)
                cr = slice(jj * Q, (jj + 1) * Q)
                nc.vector.scalar_tensor_tensor(
                    out=lhs_bot3[sl, :, cr],
                    in0=t_a3[sl],
                    scalar=-1.0,
                    in1=t_b3[sl],
                    op0=Alu.mult,
                    op1=Alu.subtract,
                )

            # ---------------- contraction + output ----------------
            vre3 = v3(v_re, T)
            vim3 = v3(v_im, T)
            for m in range(MH):
                pt = psum.tile([32, T], f32, tag="pt")
                nc.tensor.matmul(
                    pt[:], lhs_top3[:, m, :], vre3[:, m, :], start=True, stop=False
                )
                nc.tensor.matmul(
                    pt[:], lhs_bot3[:, m, :], vim3[:, m, :], start=False, stop=True
                )
                k_sb = stg.tile([32, T], f32, tag="k_sb")
                if m % 2 == 0:
                    nc.scalar.copy(k_sb[:], pt[:])
                else:
                    nc.vector.tensor_copy(k_sb[:], pt[:])
                dma_eng = nc.sync if m % 2 == 0 else nc.gpsimd
                dma_eng.dma_start(
                    out=_ap(K, m * 2 * L, [[L, 2], [T, Q], [1, T]]),
                    in_=k_sb[:],
                )

        # pin ACT order: exps first, then sins (one table load each)
        chain = exps + sins
        for prev, nxt in zip(chain, chain[1:]):
            add_dep_helper(nxt, prev, sync=False, reason="act table-set ordering")

    nc.compile()
    return nc


_NC_CACHE = {}


def kernel(log_dt, Lambda, W, L):
    assert int(L) == 2048 and log_dt.shape == (H, 2) and W.shape == (1, H, N, 2)
    if "nc" not in _NC_CACHE:
        _NC_CACHE["nc"] = build_kernel()
    nc = _NC_CACHE["nc"]

    from concourse.bass_utils import run_bass_kernel_spmd

    in_maps = [prep_core_inputs(c, log_dt, Lambda, W) for c in range(M_CORES)]
    res = run_bass_kernel_spmd(nc, in_maps, list(range(M_CORES)))
    out = np.concatenate([res.results[c]["K"] for c in range(M_CORES)], axis=0)
    return out.reshape(1, H, L).astype(np.float32)


# revision 21
# speedup vs baseline: 1.0475x; 1.0475x over previous
"""Bass/Trainium2 SPMD kernel for nn_DSSKernel (DSS: Diagonal State Space kernel).

K[c,h,l] = Re( sum_n Wc'[c,h,n] * exp(dt_Lambda[h,n] * (l - s[n])) ),  c=C=1

Strategy:
 - Shard H=512 across 8 cores (Hc=64 per core); no cross-core comms.
 - Split l = q*T + r (T=128, Q=16): S = U (x) V needs only N*Hc*(T+Q)
   transcendentals per core instead of N*Hc*L.
 - Layout: partition p = 64*j + n (j = h parity), free index m, h = 2m+j.
   Host does all input layout transforms; device DMAs are contiguous.
 - Phase args built by ACT per-m (scale = per-partition AP), phases reduced
   via exact frac trick: y = theta/2pi; frac = y - round(y) (exact, |frac|<=.5)
   then sin(2pi*frac) via ACT Sin scale; cos via frac+0.25 wrap. No Cody-Waite.
 - Contraction over n on PE with f32r single-pass matmuls, block-diagonal
   stationary tiles (h-pair per matmul pair, PSUM accumulated).
 - V pipeline chunked (CH=4) so PE/copies/DMAs overlap DVE work.
 - ACT chained: all exps, then all sins (2 table loads).
"""

import sys

import numpy as np

if "/opt/trn_rl_repo" not in sys.path:
    sys.path.insert(0, "/opt/trn_rl_repo")

import concourse.bacc as bacc
import concourse.bass as bass
import concourse.tile as tile
from concourse import mybir
from concourse.tile import add_dep_helper

f32 = mybir.dt.float32
f32r = mybir.dt.float32r
Act = mybir.ActivationFunctionType
Alu = mybir.AluOpType

M_CORES = 8
H, N, L = 512, 64, 2048
HC = H // M_CORES          # 64 h-channels per core
T = 128                    # inner block length (V)
Q = L // T                 # 16 outer blocks (U)
MH = HC // 2               # 32 h-pairs per core
CH = 4                     # chunks of the V pipeline
CM = MH // CH              # 8 h-pairs per chunk
EPS = 1e-7

TWO_PI = float(2 * np.pi)
INV_2PI = float(np.float32(1.0 / (2 * np.pi)))
MAGIC = 12582912.0         # 1.5 * 2^23: round-to-nearest for |y| < 2^22


def _ap(t, offset, pattern):
    return bass.AP(tensor=t, offset=offset, ap=[list(p) for p in pattern])


def prep_core_inputs(c, log_dt, Lambda, W):
    """Host-side shard + layout: partition p = 64*j + n, free m; h = 2m+j."""
    hs = slice(c * HC, (c + 1) * HC)
    Wc = np.asarray(W, np.float32)[0, hs]            # (HC, N, 2)
    ld = np.asarray(log_dt, np.float32)[hs]          # (HC, 2)
    lam = np.asarray(Lambda, np.float32)             # (N, 2)
    wre = Wc[:, :, 0].reshape(MH, 2, N).transpose(1, 2, 0).reshape(128, MH)
    wim = Wc[:, :, 1].reshape(MH, 2, N).transpose(1, 2, 0).reshape(128, MH)
    ldr = np.broadcast_to(
        ld[:, 0].reshape(MH, 2).T[:, None, :], (2, N, MH)
    ).reshape(128, MH)
    ldi = np.broadcast_to(
        ld[:, 1].reshape(MH, 2).T[:, None, :], (2, N, MH)
    ).reshape(128, MH)
    return {
        "Wre": np.ascontiguousarray(wre),
        "Wim": np.ascontiguousarray(wim),
        "ldt_re": np.ascontiguousarray(ldr),
        "ldt_im": np.ascontiguousarray(ldi),
        "Lam": np.ascontiguousarray(np.tile(lam, (2, 1))),  # (128, 2)
    }


def build_kernel():
    nc = bacc.Bacc()
    in_wre = nc.dram_tensor("Wre", [128, MH], f32, kind="ExternalInput")
    in_wim = nc.dram_tensor("Wim", [128, MH], f32, kind="ExternalInput")
    in_ldr = nc.dram_tensor("ldt_re", [128, MH], f32, kind="ExternalInput")
    in_ldi = nc.dram_tensor("ldt_im", [128, MH], f32, kind="ExternalInput")
    in_lam = nc.dram_tensor("Lam", [128, 2], f32, kind="ExternalInput")
    K = nc.dram_tensor("K", [HC, L], f32, kind="ExternalOutput")

    exps = []
    sins = []

    def _last(lst):
        lst.append(list(nc.all_instructions())[-1])

    with tile.TileContext(nc) as tc:
        with (
            tc.tile_pool(name="prep", bufs=1) as prep,
            tc.tile_pool(name="big", bufs=1) as big,
            tc.tile_pool(name="chk", bufs=2) as chk,
            tc.tile_pool(name="psum", bufs=8, space="PSUM") as psum,
            tc.tile_pool(name="stg", bufs=6) as stg,
        ):
            P = 128

            def v3(t, inner):
                return t[:].rearrange("p (m x) -> p m x", x=inner)

            # ---------------- input loads (all contiguous) ----------------
            lam_sb = prep.tile([P, 2], f32, tag="lam")
            nc.sync.dma_start(out=lam_sb[:], in_=in_lam[:, :])
            lam_re = lam_sb[:, 0:1]
            lam_im = lam_sb[:, 1:2]
            ldt_re = prep.tile([P, MH], f32, tag="ldt_re")
            ldt_im = prep.tile([P, MH], f32, tag="ldt_im")
            nc.sync.dma_start(out=ldt_re[:], in_=in_ldr[:, :])
            nc.sync.dma_start(out=ldt_im[:], in_=in_ldi[:, :])
            w_re = prep.tile([P, MH], f32, tag="w_re")
            w_im = prep.tile([P, MH], f32, tag="w_im")
            nc.sync.dma_start(out=w_re[:], in_=in_wre[:, :])
            nc.sync.dma_start(out=w_im[:], in_=in_wim[:, :])

            # ---------------- phase A: per-(n,h) scalars [P, MH] ----------------
            dt_re = prep.tile([P, MH], f32, tag="dt_re")
            dt_im = prep.tile([P, MH], f32, tag="dt_im")
            nc.scalar.activation(dt_re[:], ldt_re[:], Act.Exp)
            _last(exps)
            nc.scalar.activation(dt_im[:], ldt_im[:], Act.Exp)
            _last(exps)

            a_re = prep.tile([P, MH], f32, tag="a_re")
            a_imS = prep.tile([P, MH], f32, tag="a_imS")  # a_im / 2pi
            nc.vector.tensor_scalar_mul(a_re[:], dt_re[:], lam_re)
            nc.vector.tensor_scalar_mul(a_imS[:], dt_im[:], lam_im)
            nc.vector.tensor_scalar(a_imS[:], a_imS[:], INV_2PI, None, Alu.mult)

            pos = prep.tile([P, 1], f32, tag="pos")
            s1 = prep.tile([P, 1], f32, tag="s1")
            sshift = prep.tile([P, 1], f32, tag="sshift")
            nc.vector.tensor_scalar(pos[:], lam_re, 0.0, None, Alu.is_gt)
            nc.vector.tensor_scalar(s1[:], pos[:], -2.0, 1.0, Alu.mult, Alu.add)
            nc.vector.tensor_scalar_mul(sshift[:], pos[:], float(L - 1))

            an_re = prep.tile([P, MH], f32, tag="an_re")
            y1 = prep.tile([P, MH], f32, tag="y1")        # an_im / 2pi
            nc.vector.tensor_scalar_mul(an_re[:], a_re[:], s1[:])
            nc.vector.tensor_scalar_mul(y1[:], a_imS[:], s1[:])

            e1 = prep.tile([P, MH], f32, tag="e1")
            nc.scalar.activation(e1[:], an_re[:], Act.Exp)
            _last(exps)
            eL = prep.tile([P, MH], f32, tag="eL")
            nc.scalar.activation(eL[:], an_re[:], Act.Exp, scale=float(L))
            _last(exps)

            def reduce_frac(y_ap, frac_t, t_t, k_t):
                # frac = y - round(y), exact; |frac| <= 0.5
                nc.vector.tensor_scalar(t_t[:], y_ap, MAGIC, None, Alu.add)
                nc.vector.tensor_scalar(k_t[:], t_t[:], MAGIC, None, Alu.subtract)
                nc.vector.tensor_sub(frac_t[:], y_ap, k_t[:])

            tmp1 = prep.tile([P, MH], f32, tag="tmp1")
            tmp2 = prep.tile([P, MH], f32, tag="tmp2")
            fr1 = prep.tile([P, MH], f32, tag="fr1")
            reduce_frac(y1[:], fr1, tmp1, tmp2)
            fr1c = prep.tile([P, MH], f32, tag="fr1c")
            nc.vector.add_range_wrap(fr1c[:], fr1[:], 0.25, 0.5, 1.0)

            yL = prep.tile([P, MH], f32, tag="yL")
            nc.vector.tensor_scalar_mul(yL[:], fr1[:], float(L))
            frL = prep.tile([P, MH], f32, tag="frL")
            reduce_frac(yL[:], frL, tmp1, tmp2)
            frLc = prep.tile([P, MH], f32, tag="frLc")
            nc.vector.add_range_wrap(frLc[:], frL[:], 0.25, 0.5, 1.0)

            sin1 = prep.tile([P, MH], f32, tag="sin1")
            cos1 = prep.tile([P, MH], f32, tag="cos1")
            sinL = prep.tile([P, MH], f32, tag="sinL")
            cosL = prep.tile([P, MH], f32, tag="cosL")
            nc.scalar.activation(sin1[:], fr1[:], Act.Sin, scale=TWO_PI)
            _last(sins)
            nc.scalar.activation(cos1[:], fr1c[:], Act.Sin, scale=TWO_PI)
            _last(sins)
            nc.scalar.activation(sinL[:], frL[:], Act.Sin, scale=TWO_PI)
            _last(sins)
            nc.scalar.activation(cosL[:], frLc[:], Act.Sin, scale=TWO_PI)
            _last(sins)

            # num = e1*(cos1 + i sin1) - 1 ; den = eL*(cosL + i sinL) - 1
            num_re = prep.tile([P, MH], f32, tag="num_re")
            num_im = prep.tile([P, MH], f32, tag="num_im")
            nc.vector.tensor_mul(num_re[:], e1[:], cos1[:])
            nc.vector.tensor_scalar(num_re[:], num_re[:], 1.0, None, Alu.subtract)
            nc.vector.tensor_mul(num_im[:], e1[:], sin1[:])
            den_re = prep.tile([P, MH], f32, tag="den_re")
            den_im = prep.tile([P, MH], f32, tag="den_im")
            nc.vector.tensor_mul(den_re[:], eL[:], cosL[:])
            nc.vector.tensor_scalar(den_re[:], den_re[:], 1.0, None, Alu.subtract)
            nc.vector.tensor_mul(den_im[:], eL[:], sinL[:])

            # x = den * Lam ; recip = conj(x)/(|x|^2 + eps) = rr - i*ri
            x_re = prep.tile([P, MH], f32, tag="x_re")
            x_im = prep.tile([P, MH], f32, tag="x_im")
            nc.vector.tensor_scalar_mul(x_re[:], den_re[:], lam_re)
            nc.vector.tensor_scalar_mul(tmp1[:], den_im[:], lam_im)
            nc.vector.tensor_sub(x_re[:], x_re[:], tmp1[:])
            nc.vector.tensor_scalar_mul(x_im[:], den_re[:], lam_im)
            nc.vector.tensor_scalar_mul(tmp1[:], den_im[:], lam_re)
            nc.vector.tensor_add(x_im[:], x_im[:], tmp1[:])

            d2 = prep.tile([P, MH], f32, tag="d2")
            nc.vector.tensor_mul(d2[:], x_re[:], x_re[:])
            nc.vector.tensor_mul(tmp1[:], x_im[:], x_im[:])
            nc.vector.tensor_add(d2[:], d2[:], tmp1[:])
            nc.vector.tensor_scalar(d2[:], d2[:], EPS, None, Alu.add)
            inv = prep.tile([P, MH], f32, tag="inv")
            nc.vector.reciprocal(inv[:], d2[:])
            rr = prep.tile([P, MH], f32, tag="rr")
            ri = prep.tile([P, MH], f32, tag="ri")
            nc.vector.tensor_mul(rr[:], x_re[:], inv[:])
            nc.vector.tensor_mul(ri[:], x_im[:], inv[:])

            # F = num * (rr - i*ri)
            f_re = prep.tile([P, MH], f32, tag="f_re")
            f_im = prep.tile([P, MH], f32, tag="f_im")
            nc.vector.tensor_mul(f_re[:], num_re[:], rr[:])
            nc.vector.tensor_mul(tmp1[:], num_im[:], ri[:])
            nc.vector.tensor_add(f_re[:], f_re[:], tmp1[:])
            nc.vector.tensor_mul(f_im[:], num_im[:], rr[:])
            nc.vector.tensor_mul(tmp1[:], num_re[:], ri[:])
            nc.vector.tensor_sub(f_im[:], f_im[:], tmp1[:])

            # B = Wc * F
            b_re = prep.tile([P, MH], f32, tag="b_re")
            b_im = prep.tile([P, MH], f32, tag="b_im")
            nc.vector.tensor_mul(b_re[:], w_re[:], f_re[:])
            nc.vector.tensor_mul(tmp1[:], w_im[:], f_im[:])
            nc.vector.tensor_sub(b_re[:], b_re[:], tmp1[:])
            nc.vector.tensor_mul(b_im[:], w_re[:], f_im[:])
            nc.vector.tensor_mul(tmp1[:], w_im[:], f_re[:])
            nc.vector.tensor_add(b_im[:], b_im[:], tmp1[:])

            # pre-reduced a_im/2pi (integer multiples preserve frac phase)
            a_imR = prep.tile([P, MH], f32, tag="a_imR")
            reduce_frac(a_imS[:], a_imR, tmp1, tmp2)

            # ---------------- U build (full width, [P, MH*Q]) ----------------
            iota_q = big.tile([P, Q], f32, tag="iota_q")
            nc.gpsimd.iota(
                iota_q[:], pattern=[[T, Q]], channel_multiplier=0,
                allow_small_or_imprecise_dtypes=True,
            )
            tq_s = big.tile([P, Q], f32, tag="tq_s")
            nc.vector.tensor_scalar(tq_s[:], iota_q[:], sshift[:], None, Alu.subtract)

            u_arg = big.tile([P, MH * Q], f32, tag="u_arg")
            u_y = big.tile([P, MH * Q], f32, tag="u_y")
            nc.vector.tensor_tensor(
                v3(u_arg, Q), tq_s[:, None, :].broadcast_to((P, MH, Q)),
                a_re[:, :, None].broadcast_to((P, MH, Q)), Alu.mult
            )
            eu = big.tile([P, MH * Q], f32, tag="u_eu")
            nc.scalar.activation(eu[:], u_arg[:], Act.Exp)
            _last(exps)
            nc.vector.tensor_tensor(
                v3(u_y, Q), tq_s[:, None, :].broadcast_to((P, MH, Q)),
                a_imR[:, :, None].broadcast_to((P, MH, Q)), Alu.mult
            )
            u_t = big.tile([P, MH * Q], f32, tag="u_t")
            u_k = big.tile([P, MH * Q], f32, tag="u_k")
            u_fr = big.tile([P, MH * Q], f32, tag="u_fr")
            nc.vector.tensor_scalar(u_t[:], u_y[:], MAGIC, None, Alu.add)
            nc.vector.tensor_scalar(u_k[:], u_t[:], MAGIC, None, Alu.subtract)
            nc.vector.tensor_sub(u_fr[:], u_y[:], u_k[:])
            u_frc = big.tile([P, MH * Q], f32, tag="u_frc")
            nc.vector.add_range_wrap(u_frc[:], u_fr[:], 0.25, 0.5, 1.0)

            scos_u = big.tile([P, MH * Q], f32, tag="u_scos")
            ssin_u = big.tile([P, MH * Q], f32, tag="u_ssin")
            nc.scalar.activation(scos_u[:], u_frc[:], Act.Sin, scale=TWO_PI)
            _last(sins)
            nc.scalar.activation(ssin_u[:], u_fr[:], Act.Sin, scale=TWO_PI)
            _last(sins)

            ec = big.tile([P, MH * Q], f32, tag="u_ec")
            es = big.tile([P, MH * Q], f32, tag="u_es")
            nc.vector.tensor_mul(ec[:], eu[:], scos_u[:])
            nc.vector.tensor_mul(es[:], eu[:], ssin_u[:])

            # block-diagonal stationary tiles
            lhs_top = big.tile([P, MH * 2 * Q], f32r, tag="lhs_top")
            lhs_bot = big.tile([P, MH * 2 * Q], f32r, tag="lhs_bot")
            nc.vector.memset(lhs_top[:].bitcast(f32), 0.0)
            nc.vector.memset(lhs_bot[:].bitcast(f32), 0.0)
            lhs_top3 = v3(lhs_top, 2 * Q)
            lhs_bot3 = v3(lhs_bot, 2 * Q)
            ec3 = v3(ec, Q)
            es3 = v3(es, Q)
            t_a = big.tile([P, MH * Q], f32, tag="t_a")
            t_b = big.tile([P, MH * Q], f32, tag="t_b")
            t_a3 = v3(t_a, Q)
            t_b3 = v3(t_b, Q)
            nc.vector.tensor_tensor(
                t_a3, es3, b_im[:, :, None].broadcast_to((P, MH, Q)), Alu.mult
            )
            nc.vector.tensor_tensor(
                t_b3, ec3, b_re[:, :, None].broadcast_to((P, MH, Q)), Alu.mult
            )
            for jj in range(2):
                sl = slice(jj * 64, (jj + 1) * 64)
                cr = slice(jj * Q, (jj + 1) * Q)
                nc.vector.tensor_sub(lhs_top3[sl, :, cr], t_b3[sl], t_a3[sl])
            nc.vector.tensor_tensor(
                t_a3, es3, b_re[:, :, None].broadcast_to((P, MH, Q)), Alu.mult
            )
            nc.vector.tensor_tensor(
                t_b3, ec3, b_im[:, :, None].broadcast_to((P, MH, Q)), Alu.mult
            )
            for jj in range(2):
                sl = slice(jj * 64, (jj + 1) * 64)
                cr = slice(jj * Q, (jj + 1) * Q)
                nc.vector.scalar_tensor_tensor(
                    out=lhs_bot3[sl, :, cr],
                    in0=t_a3[sl],
                    scalar=-1.0,
                    in1=t_b3[sl],
                    op0=Alu.mult,
                    op1=Alu.subtract,
                )

            # ---------------- V build (per-m ACT args; chunked DVE) ----------------
            iota_t = big.tile([P, T], f32, tag="iota_t")
            nc.gpsimd.iota(
                iota_t[:], pattern=[[1, T]], channel_multiplier=0,
                allow_small_or_imprecise_dtypes=True,
            )
            ev_full = big.tile([P, MH * T], f32, tag="ev_full")
            yv_full = big.tile([P, MH * T], f32, tag="yv_full")
            ev3 = v3(ev_full, T)
            yv3 = v3(yv_full, T)
            for m in range(MH):
                nc.scalar.activation(
                    ev3[:, m, :], iota_t[:], Act.Exp, scale=a_re[:, m : m + 1]
                )
                _last(exps)
            for m in range(MH):
                # y = iota * (a_im/2pi mod 1)  (Copy supports AP scale)
                nc.scalar.mul(yv3[:, m, :], iota_t[:], a_imR[:, m : m + 1])

            for ch in range(CH):
                csl = slice(ch * CM * T, (ch + 1) * CM * T)
                v_t = chk.tile([P, CM * T], f32, tag="v_t")
                v_k = chk.tile([P, CM * T], f32, tag="v_k")
                v_fr = chk.tile([P, CM * T], f32, tag="v_fr")
                v_frc = chk.tile([P, CM * T], f32, tag="v_frc")
                nc.vector.tensor_scalar(v_t[:], yv_full[:, csl], MAGIC, None, Alu.add)
                nc.vector.tensor_scalar(v_k[:], v_t[:], MAGIC, None, Alu.subtract)
                nc.gpsimd.tensor_sub(v_fr[:], yv_full[:, csl], v_k[:])
                nc.vector.add_range_wrap(v_frc[:], v_fr[:], 0.25, 0.5, 1.0)

                scos = chk.tile([P, CM * T], f32, tag="scos")
                ssin = chk.tile([P, CM * T], f32, tag="ssin")
                nc.scalar.activation(scos[:], v_frc[:], Act.Sin, scale=TWO_PI)
                _last(sins)
                nc.scalar.activation(ssin[:], v_fr[:], Act.Sin, scale=TWO_PI)
                _last(sins)

                v_re = chk.tile([P, CM * T], f32r, tag="v_re")
                v_im = chk.tile([P, CM * T], f32r, tag="v_im")
                nc.vector.tensor_mul(v_re[:], ev_full[:, csl], scos[:])
                nc.gpsimd.tensor_mul(v_im[:], ev_full[:, csl], ssin[:])
                vre3 = v3(v_re, T)
                vim3 = v3(v_im, T)

                for mm in range(CM):
                    m = ch * CM + mm
                    pt = psum.tile([32, T], f32, tag="pt")
                    nc.tensor.matmul(
                        pt[:], lhs_top3[:, m, :], vre3[:, mm, :],
                        start=True, stop=False,
                    )
                    nc.tensor.matmul(
                        pt[:], lhs_bot3[:, m, :], vim3[:, mm, :],
                        start=False, stop=True,
                    )
                    k_sb = stg.tile([32, T], f32, tag="k_sb")
                    if m % 2 == 0:
                        nc.scalar.copy(k_sb[:], pt[:])
                    else:
                        nc.vector.tensor_copy(k_sb[:], pt[:])
                    dma_eng = nc.sync if m % 2 == 0 else nc.gpsimd
                    dma_eng.dma_start(
                        out=_ap(K, m * 2 * L, [[L, 2], [T, Q], [1, T]]),
                        in_=k_sb[:],
                    )

        # pin ACT order: exps first, then sins (one table load each)
        chain = exps + sins
        for prev, nxt in zip(chain, chain[1:]):
            add_dep_helper(nxt, prev, sync=False, reason="act table-set ordering")

    nc.compile()
    return nc


_NC_CACHE = {}


def kernel(log_dt, Lambda, W, L):
    assert int(L) == 2048 and log_dt.shape == (H, 2) and W.shape == (1, H, N, 2)
    if "nc" not in _NC_CACHE:
        _NC_CACHE["nc"] = build_kernel()
    nc = _NC_CACHE["nc"]

    from concourse.bass_utils import run_bass_kernel_spmd

    in_maps = [prep_core_inputs(c, log_dt, Lambda, W) for c in range(M_CORES)]
    res = run_bass_kernel_spmd(nc, in_maps, list(range(M_CORES)))
    out = np.concatenate([res.results[c]["K"] for c in range(M_CORES)], axis=0)
    return out.reshape(1, H, L).astype(np.float32)


# revision 22
# speedup vs baseline: 1.1256x; 1.0746x over previous
"""Bass/Trainium2 SPMD kernel for nn_DSSKernel (DSS: Diagonal State Space kernel).

K[c,h,l] = Re( sum_n Wc'[c,h,n] * exp(dt_Lambda[h,n] * (l - s[n])) ),  c=C=1

Strategy:
 - Shard H=512 across 8 cores (Hc=64 per core); no cross-core comms.
 - Split l = q*T + r (T=128, Q=16): S = U (x) V needs only N*Hc*(T+Q)
   transcendentals per core instead of N*Hc*L.
 - Layout: partition p = 64*j + n (j = h parity), free index m, h = 2m+j.
   Host does all input layout transforms; device DMAs are contiguous.
 - Phase args built by ACT per-m (scale = per-partition AP), phases reduced
   via exact frac trick: y = theta/2pi; frac = y - round(y) (exact, |frac|<=.5)
   then sin(2pi*frac) via ACT Sin scale; cos via frac+0.25 wrap. No Cody-Waite.
 - Contraction over n on PE with f32r single-pass matmuls, block-diagonal
   stationary tiles (h-pair per matmul pair, PSUM accumulated).
 - V pipeline chunked (CH=4) so PE/copies/DMAs overlap DVE work.
 - ACT chained: all exps, then all sins (2 table loads).
"""

import sys

import numpy as np

if "/opt/trn_rl_repo" not in sys.path:
    sys.path.insert(0, "/opt/trn_rl_repo")

import concourse.bacc as bacc
import concourse.bass as bass
import concourse.tile as tile
from concourse import mybir
from concourse.tile import add_dep_helper

f32 = mybir.dt.float32
f32r = mybir.dt.float32r
Act = mybir.ActivationFunctionType
Alu = mybir.AluOpType

M_CORES = 8
H, N, L = 512, 64, 2048
HC = H // M_CORES          # 64 h-channels per core
T = 128                    # inner block length (V)
Q = L // T                 # 16 outer blocks (U)
MH = HC // 2               # 32 h-pairs per core
CH = 4                     # chunks of the V pipeline
CM = MH // CH              # 8 h-pairs per chunk
EPS = 1e-7

TWO_PI = float(2 * np.pi)
INV_2PI = float(np.float32(1.0 / (2 * np.pi)))
MAGIC = 12582912.0         # 1.5 * 2^23: round-to-nearest for |y| < 2^22


def _ap(t, offset, pattern):
    return bass.AP(tensor=t, offset=offset, ap=[list(p) for p in pattern])


def prep_core_inputs(c, log_dt, Lambda, W):
    """Host-side shard + layout: partition p = 64*j + n, free m; h = 2m+j."""
    hs = slice(c * HC, (c + 1) * HC)
    Wc = np.asarray(W, np.float32)[0, hs]            # (HC, N, 2)
    ld = np.asarray(log_dt, np.float32)[hs]          # (HC, 2)
    lam = np.asarray(Lambda, np.float32)             # (N, 2)
    wre = Wc[:, :, 0].reshape(MH, 2, N).transpose(1, 2, 0).reshape(128, MH)
    wim = Wc[:, :, 1].reshape(MH, 2, N).transpose(1, 2, 0).reshape(128, MH)
    ldr = np.broadcast_to(
        ld[:, 0].reshape(MH, 2).T[:, None, :], (2, N, MH)
    ).reshape(128, MH)
    ldi = np.broadcast_to(
        ld[:, 1].reshape(MH, 2).T[:, None, :], (2, N, MH)
    ).reshape(128, MH)
    return {
        "Wre": np.ascontiguousarray(wre),
        "Wim": np.ascontiguousarray(wim),
        "ldt_re": np.ascontiguousarray(ldr),
        "ldt_im": np.ascontiguousarray(ldi),
        "Lam": np.ascontiguousarray(np.tile(lam, (2, 1))),  # (128, 2)
    }


def build_kernel():
    nc = bacc.Bacc()
    in_wre = nc.dram_tensor("Wre", [128, MH], f32, kind="ExternalInput")
    in_wim = nc.dram_tensor("Wim", [128, MH], f32, kind="ExternalInput")
    in_ldr = nc.dram_tensor("ldt_re", [128, MH], f32, kind="ExternalInput")
    in_ldi = nc.dram_tensor("ldt_im", [128, MH], f32, kind="ExternalInput")
    in_lam = nc.dram_tensor("Lam", [128, 2], f32, kind="ExternalInput")
    K = nc.dram_tensor("K", [HC, L], f32, kind="ExternalOutput")

    exps = []
    sins = []

    def _last(lst):
        lst.append(list(nc.all_instructions())[-1])

    with tile.TileContext(nc) as tc:
        with (
            tc.tile_pool(name="prep", bufs=1) as prep,
            tc.tile_pool(name="big", bufs=1) as big,
            tc.tile_pool(name="chk", bufs=2) as chk,
            tc.tile_pool(name="psum", bufs=8, space="PSUM") as psum,
            tc.tile_pool(name="stg", bufs=6) as stg,
        ):
            P = 128

            def v3(t, inner):
                return t[:].rearrange("p (m x) -> p m x", x=inner)

            # ---------------- input loads (all contiguous) ----------------
            lam_sb = prep.tile([P, 2], f32, tag="lam")
            nc.sync.dma_start(out=lam_sb[:], in_=in_lam[:, :])
            lam_re = lam_sb[:, 0:1]
            lam_im = lam_sb[:, 1:2]
            ldt_re = prep.tile([P, MH], f32, tag="ldt_re")
            ldt_im = prep.tile([P, MH], f32, tag="ldt_im")
            nc.sync.dma_start(out=ldt_re[:], in_=in_ldr[:, :])
            nc.sync.dma_start(out=ldt_im[:], in_=in_ldi[:, :])
            w_re = prep.tile([P, MH], f32, tag="w_re")
            w_im = prep.tile([P, MH], f32, tag="w_im")
            nc.sync.dma_start(out=w_re[:], in_=in_wre[:, :])
            nc.sync.dma_start(out=w_im[:], in_=in_wim[:, :])

            # ---------------- phase A: per-(n,h) scalars [P, MH] ----------------
            dt_re = prep.tile([P, MH], f32, tag="dt_re")
            dt_im = prep.tile([P, MH], f32, tag="dt_im")
            nc.scalar.activation(dt_re[:], ldt_re[:], Act.Exp)
            _last(exps)
            nc.scalar.activation(dt_im[:], ldt_im[:], Act.Exp)
            _last(exps)

            a_re = prep.tile([P, MH], f32, tag="a_re")
            a_imS = prep.tile([P, MH], f32, tag="a_imS")  # a_im / 2pi
            nc.vector.tensor_scalar_mul(a_re[:], dt_re[:], lam_re)
            nc.vector.tensor_scalar_mul(a_imS[:], dt_im[:], lam_im)
            nc.vector.tensor_scalar(a_imS[:], a_imS[:], INV_2PI, None, Alu.mult)

            pos = prep.tile([P, 1], f32, tag="pos")
            s1 = prep.tile([P, 1], f32, tag="s1")
            sshift = prep.tile([P, 1], f32, tag="sshift")
            nc.vector.tensor_scalar(pos[:], lam_re, 0.0, None, Alu.is_gt)
            nc.vector.tensor_scalar(s1[:], pos[:], -2.0, 1.0, Alu.mult, Alu.add)
            nc.vector.tensor_scalar_mul(sshift[:], pos[:], float(L - 1))

            an_re = prep.tile([P, MH], f32, tag="an_re")
            y1 = prep.tile([P, MH], f32, tag="y1")        # an_im / 2pi
            nc.vector.tensor_scalar_mul(an_re[:], a_re[:], s1[:])
            nc.vector.tensor_scalar_mul(y1[:], a_imS[:], s1[:])

            e1 = prep.tile([P, MH], f32, tag="e1")
            nc.scalar.activation(e1[:], an_re[:], Act.Exp)
            _last(exps)
            eL = prep.tile([P, MH], f32, tag="eL")
            nc.scalar.activation(eL[:], an_re[:], Act.Exp, scale=float(L))
            _last(exps)

            def reduce_frac(y_ap, frac_t, t_t, k_t):
                # frac = y - round(y), exact; |frac| <= 0.5
                nc.vector.tensor_scalar(t_t[:], y_ap, MAGIC, None, Alu.add)
                nc.vector.tensor_scalar(k_t[:], t_t[:], MAGIC, None, Alu.subtract)
                nc.vector.tensor_sub(frac_t[:], y_ap, k_t[:])

            tmp1 = prep.tile([P, MH], f32, tag="tmp1")
            tmp2 = prep.tile([P, MH], f32, tag="tmp2")
            fr1 = prep.tile([P, MH], f32, tag="fr1")
            reduce_frac(y1[:], fr1, tmp1, tmp2)
            fr1c = prep.tile([P, MH], f32, tag="fr1c")
            nc.vector.add_range_wrap(fr1c[:], fr1[:], 0.25, 0.5, 1.0)

            yL = prep.tile([P, MH], f32, tag="yL")
            nc.vector.tensor_scalar_mul(yL[:], fr1[:], float(L))
            frL = prep.tile([P, MH], f32, tag="frL")
            reduce_frac(yL[:], frL, tmp1, tmp2)
            frLc = prep.tile([P, MH], f32, tag="frLc")
            nc.vector.add_range_wrap(frLc[:], frL[:], 0.25, 0.5, 1.0)

            sin1 = prep.tile([P, MH], f32, tag="sin1")
            cos1 = prep.tile([P, MH], f32, tag="cos1")
            sinL = prep.tile([P, MH], f32, tag="sinL")
            cosL = prep.tile([P, MH], f32, tag="cosL")
            nc.scalar.activation(sin1[:], fr1[:], Act.Sin, scale=TWO_PI)
            _last(sins)
            nc.scalar.activation(cos1[:], fr1c[:], Act.Sin, scale=TWO_PI)
            _last(sins)
            nc.scalar.activation(sinL[:], frL[:], Act.Sin, scale=TWO_PI)
            _last(sins)
            nc.scalar.activation(cosL[:], frLc[:], Act.Sin, scale=TWO_PI)
            _last(sins)

            # num = e1*(cos1 + i sin1) - 1 ; den = eL*(cosL + i sinL) - 1
            num_re = prep.tile([P, MH], f32, tag="num_re")
            num_im = prep.tile([P, MH], f32, tag="num_im")
            nc.vector.tensor_mul(num_re[:], e1[:], cos1[:])
            nc.vector.tensor_scalar(num_re[:], num_re[:], 1.0, None, Alu.subtract)
            nc.vector.tensor_mul(num_im[:], e1[:], sin1[:])
            den_re = prep.tile([P, MH], f32, tag="den_re")
            den_im = prep.tile([P, MH], f32, tag="den_im")
            nc.vector.tensor_mul(den_re[:], eL[:], cosL[:])
            nc.vector.tensor_scalar(den_re[:], den_re[:], 1.0, None, Alu.subtract)
            nc.vector.tensor_mul(den_im[:], eL[:], sinL[:])

            # x = den * Lam ; recip = conj(x)/(|x|^2 + eps) = rr - i*ri
            x_re = prep.tile([P, MH], f32, tag="x_re")
            x_im = prep.tile([P, MH], f32, tag="x_im")
            nc.vector.tensor_scalar_mul(x_re[:], den_re[:], lam_re)
            nc.vector.tensor_scalar_mul(tmp1[:], den_im[:], lam_im)
            nc.vector.tensor_sub(x_re[:], x_re[:], tmp1[:])
            nc.vector.tensor_scalar_mul(x_im[:], den_re[:], lam_im)
            nc.vector.tensor_scalar_mul(tmp1[:], den_im[:], lam_re)
            nc.vector.tensor_add(x_im[:], x_im[:], tmp1[:])

            d2 = prep.tile([P, MH], f32, tag="d2")
            nc.vector.tensor_mul(d2[:], x_re[:], x_re[:])
            nc.vector.tensor_mul(tmp1[:], x_im[:], x_im[:])
            nc.vector.tensor_add(d2[:], d2[:], tmp1[:])
            nc.vector.tensor_scalar(d2[:], d2[:], EPS, None, Alu.add)
            inv = prep.tile([P, MH], f32, tag="inv")
            nc.vector.reciprocal(inv[:], d2[:])
            rr = prep.tile([P, MH], f32, tag="rr")
            ri = prep.tile([P, MH], f32, tag="ri")
            nc.vector.tensor_mul(rr[:], x_re[:], inv[:])
            nc.vector.tensor_mul(ri[:], x_im[:], inv[:])

            # F = num * (rr - i*ri)
            f_re = prep.tile([P, MH], f32, tag="f_re")
            f_im = prep.tile([P, MH], f32, tag="f_im")
            nc.vector.tensor_mul(f_re[:], num_re[:], rr[:])
            nc.vector.tensor_mul(tmp1[:], num_im[:], ri[:])
            nc.vector.tensor_add(f_re[:], f_re[:], tmp1[:])
            nc.vector.tensor_mul(f_im[:], num_im[:], rr[:])
            nc.vector.tensor_mul(tmp1[:], num_re[:], ri[:])
            nc.vector.tensor_sub(f_im[:], f_im[:], tmp1[:])

            # B = Wc * F
            b_re = prep.tile([P, MH], f32, tag="b_re")
            b_im = prep.tile([P, MH], f32, tag="b_im")
            nc.vector.tensor_mul(b_re[:], w_re[:], f_re[:])
            nc.vector.tensor_mul(tmp1[:], w_im[:], f_im[:])
            nc.vector.tensor_sub(b_re[:], b_re[:], tmp1[:])
            nc.vector.tensor_mul(b_im[:], w_re[:], f_im[:])
            nc.vector.tensor_mul(tmp1[:], w_im[:], f_re[:])
            nc.vector.tensor_add(b_im[:], b_im[:], tmp1[:])

            # pre-reduced a_im/2pi (integer multiples preserve frac phase)
            a_imR = prep.tile([P, MH], f32, tag="a_imR")
            reduce_frac(a_imS[:], a_imR, tmp1, tmp2)

            # ---------------- U build (full width, [P, MH*Q]) ----------------
            iota_q = big.tile([P, Q], f32, tag="iota_q")
            nc.gpsimd.iota(
                iota_q[:], pattern=[[T, Q]], channel_multiplier=0,
                allow_small_or_imprecise_dtypes=True,
            )
            tq_s = big.tile([P, Q], f32, tag="tq_s")
            nc.vector.tensor_scalar(tq_s[:], iota_q[:], sshift[:], None, Alu.subtract)

            u_arg = big.tile([P, MH * Q], f32, tag="u_arg")
            u_y = big.tile([P, MH * Q], f32, tag="u_y")
            nc.vector.tensor_tensor(
                v3(u_arg, Q), tq_s[:, None, :].broadcast_to((P, MH, Q)),
                a_re[:, :, None].broadcast_to((P, MH, Q)), Alu.mult
            )
            eu = big.tile([P, MH * Q], f32, tag="u_eu")
            nc.scalar.activation(eu[:], u_arg[:], Act.Exp)
            _last(exps)
            nc.vector.tensor_tensor(
                v3(u_y, Q), tq_s[:, None, :].broadcast_to((P, MH, Q)),
                a_imR[:, :, None].broadcast_to((P, MH, Q)), Alu.mult
            )
            u_t = big.tile([P, MH * Q], f32, tag="u_t")
            u_k = big.tile([P, MH * Q], f32, tag="u_k")
            u_fr = big.tile([P, MH * Q], f32, tag="u_fr")
            nc.vector.tensor_scalar(u_t[:], u_y[:], MAGIC, None, Alu.add)
            nc.vector.tensor_scalar(u_k[:], u_t[:], MAGIC, None, Alu.subtract)
            nc.vector.tensor_sub(u_fr[:], u_y[:], u_k[:])
            u_frc = big.tile([P, MH * Q], f32, tag="u_frc")
            nc.vector.add_range_wrap(u_frc[:], u_fr[:], 0.25, 0.5, 1.0)

            scos_u = big.tile([P, MH * Q], f32, tag="u_scos")
            ssin_u = big.tile([P, MH * Q], f32, tag="u_ssin")
            nc.scalar.activation(scos_u[:], u_frc[:], Act.Sin, scale=TWO_PI)
            _last(sins)
            nc.scalar.activation(ssin_u[:], u_fr[:], Act.Sin, scale=TWO_PI)
            _last(sins)

            ec = big.tile([P, MH * Q], f32, tag="u_ec")
            es = big.tile([P, MH * Q], f32, tag="u_es")
            nc.vector.tensor_mul(ec[:], eu[:], scos_u[:])
            nc.vector.tensor_mul(es[:], eu[:], ssin_u[:])

            # block-diagonal stationary tiles
            lhs_top = big.tile([P, MH * 2 * Q], f32r, tag="lhs_top")
            lhs_bot = big.tile([P, MH * 2 * Q], f32r, tag="lhs_bot")
            nc.vector.memset(lhs_top[:].bitcast(f32), 0.0)
            nc.vector.memset(lhs_bot[:].bitcast(f32), 0.0)
            lhs_top3 = v3(lhs_top, 2 * Q)
            lhs_bot3 = v3(lhs_bot, 2 * Q)
            ec3 = v3(ec, Q)
            es3 = v3(es, Q)
            t_a = big.tile([P, MH * Q], f32, tag="t_a")
            t_b = big.tile([P, MH * Q], f32, tag="t_b")
            t_a3 = v3(t_a, Q)
            t_b3 = v3(t_b, Q)
            nc.vector.tensor_tensor(
                t_a3, es3, b_im[:, :, None].broadcast_to((P, MH, Q)), Alu.mult
            )
            nc.vector.tensor_tensor(
                t_b3, ec3, b_re[:, :, None].broadcast_to((P, MH, Q)), Alu.mult
            )
            for jj in range(2):
                sl = slice(jj * 64, (jj + 1) * 64)
                cr = slice(jj * Q, (jj + 1) * Q)
                nc.vector.tensor_sub(lhs_top3[sl, :, cr], t_b3[sl], t_a3[sl])
            nc.vector.tensor_tensor(
                t_a3, es3, b_re[:, :, None].broadcast_to((P, MH, Q)), Alu.mult
            )
            nc.vector.tensor_tensor(
                t_b3, ec3, b_im[:, :, None].broadcast_to((P, MH, Q)), Alu.mult
            )
            for jj in range(2):
                sl = slice(jj * 64, (jj + 1) * 64)
                cr = slice(jj * Q, (jj + 1) * Q)
                nc.vector.scalar_tensor_tensor(
                    out=lhs_bot3[sl, :, cr],
                    in0=t_a3[sl],
                    scalar=-1.0,
                    in1=t_b3[sl],
                    op0=Alu.mult,
                    op1=Alu.subtract,
                )

            # ---------------- V build (per-m ACT args; chunked DVE) ----------------
            iota_t = big.tile([P, T], f32, tag="iota_t")
            nc.gpsimd.iota(
                iota_t[:], pattern=[[1, T]], channel_multiplier=0,
                allow_small_or_imprecise_dtypes=True,
            )
            ev_full = big.tile([P, MH * T], f32, tag="ev_full")
            yv_full = big.tile([P, MH * T], f32, tag="yv_full")
            v_arg = big.tile([P, MH * T], f32, tag="v_arg")
            nc.vector.tensor_tensor(
                v3(v_arg, T), iota_t[:, None, :].broadcast_to((P, MH, T)),
                a_re[:, :, None].broadcast_to((P, MH, T)), Alu.mult
            )
            nc.scalar.activation(ev_full[:], v_arg[:], Act.Exp)
            _last(exps)
            nc.vector.tensor_tensor(
                v3(yv_full, T), iota_t[:, None, :].broadcast_to((P, MH, T)),
                a_imR[:, :, None].broadcast_to((P, MH, T)), Alu.mult
            )

            for ch in range(CH):
                csl = slice(ch * CM * T, (ch + 1) * CM * T)
                v_t = chk.tile([P, CM * T], f32, tag="v_t")
                v_k = chk.tile([P, CM * T], f32, tag="v_k")
                v_fr = chk.tile([P, CM * T], f32, tag="v_fr")
                v_frc = chk.tile([P, CM * T], f32, tag="v_frc")
                nc.vector.tensor_scalar(v_t[:], yv_full[:, csl], MAGIC, None, Alu.add)
                nc.vector.tensor_scalar(v_k[:], v_t[:], MAGIC, None, Alu.subtract)
                nc.gpsimd.tensor_sub(v_fr[:], yv_full[:, csl], v_k[:])
                nc.vector.add_range_wrap(v_frc[:], v_fr[:], 0.25, 0.5, 1.0)

                scos = chk.tile([P, CM * T], f32, tag="scos")
                ssin = chk.tile([P, CM * T], f32, tag="ssin")
                nc.scalar.activation(scos[:], v_frc[:], Act.Sin, scale=TWO_PI)
                _last(sins)
                nc.scalar.activation(ssin[:], v_fr[:], Act.Sin, scale=TWO_PI)
                _last(sins)

                v_re = chk.tile([P, CM * T], f32r, tag="v_re")
                v_im = chk.tile([P, CM * T], f32r, tag="v_im")
                nc.vector.tensor_mul(v_re[:], ev_full[:, csl], scos[:])
                nc.vector.tensor_mul(v_im[:], ev_full[:, csl], ssin[:])
                vre3 = v3(v_re, T)
                vim3 = v3(v_im, T)

                for mm in range(CM):
                    m = ch * CM + mm
                    pt = psum.tile([32, T], f32, tag="pt")
                    nc.tensor.matmul(
                        pt[:], lhs_top3[:, m, :], vre3[:, mm, :],
                        start=True, stop=False,
                    )
                    nc.tensor.matmul(
                        pt[:], lhs_bot3[:, m, :], vim3[:, mm, :],
                        start=False, stop=True,
                    )
                    k_sb = stg.tile([32, T], f32, tag="k_sb")
                    nc.scalar.copy(k_sb[:], pt[:])
                    dma_eng = nc.sync if m % 2 == 0 else nc.gpsimd
                    dma_eng.dma_start(
                        out=_ap(K, m * 2 * L, [[L, 2], [T, Q], [1, T]]),
                        in_=k_sb[:],
                    )

        # pin ACT order: exps first, then sins (one table load each)
        chain = exps + sins
        for prev, nxt in zip(chain, chain[1:]):
            add_dep_helper(nxt, prev, sync=False, reason="act table-set ordering")

    nc.compile()
    return nc


_NC_CACHE = {}


def kernel(log_dt, Lambda, W, L):
    assert int(L) == 2048 and log_dt.shape == (H, 2) and W.shape == (1, H, N, 2)
    if "nc" not in _NC_CACHE:
        _NC_CACHE["nc"] = build_kernel()
    nc = _NC_CACHE["nc"]

    from concourse.bass_utils import run_bass_kernel_spmd

    in_maps = [prep_core_inputs(c, log_dt, Lambda, W) for c in range(M_CORES)]
    res = run_bass_kernel_spmd(nc, in_maps, list(range(M_CORES)))
    out = np.concatenate([res.results[c]["K"] for c in range(M_CORES)], axis=0)
    return out.reshape(1, H, L).astype(np.float32)
